# revision 1
# baseline (speedup 1.0000x reference)
"""Multi-head attention (B=4, S=2048, D=1024, H=16, Dk=64) on 8 trn2 NeuronCores.

Sharding: core c handles batch b = c//2 and head-half hh = c%2 (8 heads = 512
channels).  No collectives: each core produces a partial output projection
(sum over its 8 heads); the host adds the two partials per batch plus b_o.

Per-core kernel (fp16 operands, f32 PSUM accumulation), software-pipelined:
  - Prologue projects only K^T[0] + Q^T[0] block 0 so the exp stream starts
    immediately; V and the other K^T/Q^T projections are woven into the
    attention windows as deadline-tagged fillers.
  - Attention loops n (t_q block of 512) outer, j (head pair) inner.  Per
    window (n, j): score tiles S^T [t_k=128, 512] (K=64 matmuls) in groups
    of 3 units through a 2-deep PSUM ring, exp on ScalarE (scale=1/8
    folded), and attn*V in the TRANSPOSED orientation out[t_q=128, 65]
    (col 64 = rowsum via a fused ones-column in V) EAGERLY, two exp-groups
    behind, so PE never drains while ACT works and vice versa.
  - All 8 attn*V accumulators (4 t_q chunks x 2 heads) live in just 2 PSUM
    banks: 4 packed per bank; the first matmul per bank uses start=True
    (hardware zeroes the whole 2KB zero-region) and the rest accumulate
    with start=False.
  - Softmax normalize = per-partition reciprocal + tensor_scalar multiply.
  - O [t_q, ch] -> O^T via XBAR DMA transpose (runs on the DMA engines).
  - Projection / output-projection PSUM tiles share the score ring (views
    into the same slots), so everything fits in 8 PSUM banks.
  - K^T/Q^T-block/output-projection work items are emitted as deadline-
    tagged fillers inside attention windows to soak up PE slack.
"""

import sys
from collections import deque

import numpy as np

if "/opt/trn_rl_repo" not in sys.path:
    sys.path.insert(0, "/opt/trn_rl_repo")

import concourse.bass as bass
import concourse.tile as tile
from concourse import mybir
from concourse.bass_utils import run_bass_kernel_spmd
import concourse.bass_utils as _bass_utils
import concourse.bass2jax as _bass2jax


def _legalize_bir_json(bir_json):
    """Walrus (this toolchain's codegen) encodes at most ONE attached sync
    wait per TPB instruction; Tile emits instructions with several.  Hoist
    all but the last wait of each non-DMA instruction into standalone
    EventSemaphore instructions right before it (same engine, so stream
    order preserves the wait semantics).  DMA waits ride in queue
    descriptors and are left untouched."""
    import json as _json

    d = _json.loads(bir_json)
    n = 0
    for fn in d.get("functions", []):
        for blk in fn.get("blocks", []):
            out = []
            for inst in blk.get("instructions", []):
                si = inst.get("sync_info")
                if si and si.get("on_wait") and len(si["on_wait"]) > 1:
                    waits = si["on_wait"]
                    for w in waits[:-1]:
                        n += 1
                        out.append(
                            {
                                "debug": inst.get("debug"),
                                "engine": inst["engine"],
                                "ins": [],
                                "outs": [],
                                "name": f"{inst['name']}-hw{n}",
                                "opcode": "EventSemaphore",
                                "sync_info": {"on_update": [], "on_wait": [w]},
                            }
                        )
                    si["on_wait"] = [waits[-1]]
                out.append(inst)
            blk["instructions"] = out
    return _json.dumps(d).encode()


_orig_compile_bir_kernel = _bass_utils.compile_bir_kernel


def _patched_compile_bir_kernel(bir_json, tmpdir, neff_name="file.neff"):
    return _orig_compile_bir_kernel(_legalize_bir_json(bir_json), tmpdir, neff_name)


_bass_utils.compile_bir_kernel = _patched_compile_bir_kernel
_bass2jax.compile_bir_kernel = _patched_compile_bir_kernel

F16 = mybir.dt.float16
F32 = mybir.dt.float32
EXP = mybir.ActivationFunctionType.Exp
PSUM = bass.MemorySpace.PSUM

B = 4          # batches (full problem)
S = 2048       # sequence length
D = 1024       # d_model
CH = 512       # channels per core (8 heads x 64)
NH = 8         # heads per core
DK = 64        # head dim
NDM = 8        # d_model chunks of 128
NCI = 4        # channel chunks of 128 (head pairs)
QB = 512       # t_q block width
NQB = S // QB  # 4
NKC = S // 128  # 16 t_k chunks
EXPB = 3       # score units per exp batch
NU = 2 * NKC   # 32 (t_k chunk, head) units per (j, n)
NG = (NU + EXPB - 1) // EXPB  # 11 exp groups
OV_DEFER = 2   # attn*V trails exp by this many groups
N_CORES = 8

_NC_CACHE = None


class _Fillers:
    """Deadline-tagged PE work items woven into attention windows.

    `due` = index of the window by whose END the item must be emitted.
    Window w calls drain_due(w - 1) at its start and drain_due(w) before its
    final attn*V flush; pop_one() opportunistically drains FIFO at slack
    slots in between."""

    def __init__(self):
        self.q = deque()  # (due_window, emit_fn)

    def push(self, due, fn):
        self.q.append((due, fn))

    def drain_due(self, w):
        while self.q and self.q[0][0] <= w:
            self.q.popleft()[1]()

    def pop_one(self):
        if self.q:
            self.q.popleft()[1]()

    def drain_all(self):
        while self.q:
            self.q.popleft()[1]()


def _emit(tc, xqT, xkT, xvT, wqT, wkT, wvT, woT, bq, bk, bv, y):
    nc = tc.nc

    with (
        tc.tile_pool(name="persist", bufs=1) as persist,
        tc.tile_pool(name="wp", bufs=1) as wp,
        tc.tile_pool(name="xkp", bufs=4) as xkp,
        tc.tile_pool(name="xqp", bufs=2) as xqp,
        tc.tile_pool(name="xvp", bufs=2) as xvp,
        tc.tile_pool(name="bigp", bufs=2, space=PSUM) as bigp,
        tc.tile_pool(name="ovp", bufs=2, space=PSUM) as ovp,
        tc.tile_pool(name="ptp", bufs=NG + 2) as ptp,
        tc.tile_pool(name="o2p", bufs=4) as o2p,
        tc.tile_pool(name="rrp", bufs=4) as rrp,
        tc.tile_pool(name="yp", bufs=2) as yp,
    ):
        QT = [persist.tile([128, S], F16, tag=f"qt{i}", name=f"qt{i}") for i in range(NCI)]
        KT = [persist.tile([128, S], F16, tag=f"kt{i}", name=f"kt{i}") for i in range(NCI)]
        V = [persist.tile([128, NH, DK + 1], F16, tag=f"v{t}", name=f"v{t}") for t in range(NKC)]
        OT = [persist.tile([128, NKC, 128], F16, tag=f"ot{i}", name=f"ot{i}") for i in range(NCI)]

        wo_sb = persist.tile([128, NCI, D], F16, tag="wo", name="wo")
        bq_sb = persist.tile([128, NCI], F32, tag="bq", name="bq")
        bk_sb = persist.tile([128, NCI], F32, tag="bk", name="bk")
        bv_bc = persist.tile([128, NH, DK + 1], F32, tag="bvb", name="bvb")

        wk_sb = wp.tile([128, NDM, CH], F16, tag="wk", name="wk")
        wq_sb = wp.tile([128, NDM, CH], F16, tag="wq", name="wq")
        wv_sb = wp.tile([128, NDM, CH], F16, tag="wv", name="wv")

        # ---- input loads (SP queue order == issue order; must match the
        # prologue's consumption order: K-proj inputs, Q block 0, V's, wo) ----
        xk_rr = xkT[:].rearrange("(n p) s -> p n s", p=128)
        xq_rr = xqT[:].rearrange("(n p) s -> p n s", p=128)
        xv_rr = xvT[:].rearrange("(n p) s -> p n s", p=128)

        nc.sync.dma_start(out=wk_sb, in_=wkT[:].rearrange("(n p) c -> p n c", p=128))
        nc.sync.dma_start(out=bk_sb, in_=bk[:].rearrange("(n p) -> p n", p=128))

        xk_blk = []
        for b in range(NQB):
            t = xkp.tile([128, NDM, QB], F16, tag="xk", name=f"xk{b}")
            nc.sync.dma_start(out=t, in_=xk_rr[:, :, QB * b : QB * (b + 1)])
            xk_blk.append(t)
            if b == 0:
                nc.sync.dma_start(
                    out=wq_sb, in_=wqT[:].rearrange("(n p) c -> p n c", p=128)
                )
                nc.sync.dma_start(out=bq_sb, in_=bq[:].rearrange("(n p) -> p n", p=128))

        xq_blk = [xqp.tile([128, NDM, QB], F16, tag="xq", name=f"xq{b}") for b in range(NQB)]
        nc.sync.dma_start(out=xq_blk[0], in_=xq_rr[:, :, 0:QB])

        nc.sync.dma_start(out=wv_sb, in_=wvT[:].rearrange("(n p) c -> p n c", p=128))
        bv_r = bv[:].rearrange("(h d) -> h d", h=NH)
        nc.sync.dma_start(out=bv_bc[:, :, 0:DK], in_=bv_r.partition_broadcast(128))
        nc.vector.memset(bv_bc[:, :, DK : DK + 1], 0.0)

        xv_blk = []
        for b in range(NQB):
            t = xvp.tile([128, NDM, QB], F16, tag="xv", name=f"xv{b}")
            nc.sync.dma_start(out=t, in_=xv_rr[:, :, QB * b : QB * (b + 1)])
            xv_blk.append(t)

        nc.sync.dma_start(out=wo_sb, in_=woT[:].rearrange("(n p) d -> p n d", p=128))

        # ---- emission helpers ----
        def big_half():
            """A [128, 512] f32 PSUM view from the shared score ring."""
            t = bigp.tile([128, EXPB * QB], F32, tag="s", name="s")
            return t, t[:, 0:QB]

        def proj_kq(dst, w_sb, bias_sb, j, blk, x_t):
            _, ps = big_half()
            for k in range(NDM):
                nc.tensor.matmul(
                    ps,
                    w_sb[:, k, 128 * j : 128 * (j + 1)],
                    x_t[:, k, :],
                    start=(k == 0),
                    stop=(k == NDM - 1),
                )
            nc.vector.tensor_scalar_add(
                out=dst[:, QB * blk : QB * (blk + 1)],
                in0=ps,
                scalar1=bias_sb[:, j : j + 1],
            )

        def proj_v(tb):
            x_t = xv_blk[tb // 4]
            c = tb % 4
            _, ps = big_half()
            for k in range(NDM):
                nc.tensor.matmul(
                    ps,
                    x_t[:, k, 128 * c : 128 * (c + 1)],
                    wv_sb[:, k, :],
                    start=(k == 0),
                    stop=(k == NDM - 1),
                )
            psv = ps.rearrange("p (h d) -> p h d", h=NH)
            nc.vector.tensor_add(out=V[tb][:, :, 0:DK], in0=psv, in1=bv_bc[:, :, 0:DK])
            nc.vector.memset(V[tb][:, :, DK : DK + 1], 1.0)

        y_r = y[:].rearrange("(a p) d -> a p d", p=128)

        def units_of(g):
            u0 = EXPB * g
            return [divmod(u, 2) for u in range(u0, min(u0 + EXPB, NU))]

        def attn_window(n, j, fillers, ov_defer=OV_DEFER, pops_per_slot=1,
                        defer_finale=False, prev_finale=None):
            w = 4 * n + j
            fillers.drain_due(w - 1)
            ov = [
                ovp.tile([128, NQB, DK + 1], F32, tag="ov", name=f"ov{h}")
                for h in range(2)
            ]
            started = [False, False]
            pt_tiles = []

            def emit_ov(g):
                for du, (tb, h) in enumerate(units_of(g)):
                    for c in range(NQB):
                        nc.tensor.matmul(
                            ov[h][:, c, :],
                            pt_tiles[g][:, QB * du + 128 * c : QB * du + 128 * (c + 1)],
                            V[tb][:, 2 * j + h, :],
                            start=(not started[h]) and (c == 0),
                            stop=(tb == NKC - 1),
                            skip_group_check=True,
                        )
                    started[h] = True

            for g in range(NG):
                us = units_of(g)
                ps = bigp.tile([128, EXPB * QB], F32, tag="s", name="s")
                for du, (tb, h) in enumerate(us):
                    nc.tensor.matmul(
                        ps[:, QB * du : QB * (du + 1)],
                        KT[j][64 * h : 64 * (h + 1), 128 * tb : 128 * (tb + 1)],
                        QT[j][64 * h : 64 * (h + 1), QB * n : QB * (n + 1)],
                        start=True,
                        stop=True,
                    )
                pt = ptp.tile([128, EXPB * QB], F16, tag="pt", name="pt")
                nc.scalar.activation(
                    out=pt[:, : QB * len(us)],
                    in_=ps[:, : QB * len(us)],
                    func=EXP,
                    scale=0.125,
                )
                pt_tiles.append(pt)
                if g == 1 and prev_finale is not None:
                    prev_finale()
                if g >= ov_defer:
                    emit_ov(g - ov_defer)
                if g >= 2:
                    for _ in range(pops_per_slot):
                        fillers.pop_one()

            def finale():
                fillers.drain_due(w)
                for g in range(max(NG - ov_defer, 0), NG):
                    emit_ov(g)
                r4 = []
                for h in range(2):
                    r = rrp.tile([128, NQB], F32, tag="r", name="r")
                    nc.vector.reciprocal(r, ov[h][:, :, DK])
                    r4.append(r)
                for c in range(NQB):
                    o2 = o2p.tile([128, 128], F16, tag="o2", name="o2")
                    for h in range(2):
                        nc.vector.tensor_scalar_mul(
                            out=o2[:, DK * h : DK * (h + 1)],
                            in0=ov[h][:, c, 0:DK],
                            scalar1=r4[h][:, c : c + 1],
                        )
                    nc.sync.dma_start_transpose(OT[j][:, NQB * n + c, :], o2)

            if defer_finale:
                return finale
            finale()
            return None

        def out_proj_m(m):
            y_sb = yp.tile([128, D], F16, tag="y", name="y")
            for half in range(2):
                _, ps = big_half()
                for jj in range(NCI):
                    nc.tensor.matmul(
                        ps,
                        OT[jj][:, m, :],
                        wo_sb[:, jj, QB * half : QB * (half + 1)],
                        start=(jj == 0),
                        stop=(jj == NCI - 1),
                    )
                nc.vector.tensor_copy(y_sb[:, QB * half : QB * (half + 1)], ps)
                # fire each half's store as soon as its copy lands, so the
                # final row's DMA doesn't wait for the whole tile
                nc.sync.dma_start(
                    out=y_r[m][:, QB * half : QB * (half + 1)],
                    in_=y_sb[:, QB * half : QB * (half + 1)],
                )

        # ---- prologue: K^T[0] fully + Q^T[0] block 0 only, so the exp
        # stream starts immediately; V and the other K^T/Q^T projections are
        # woven into the first windows as deadline-tagged fillers. ----
        for blk in range(NQB):
            proj_kq(KT[0], wk_sb, bk_sb, 0, blk, xk_blk[blk])
        proj_kq(QT[0], wq_sb, bq_sb, 0, 0, xq_blk[0])

        fillers = _Fillers()
        for tb in range(NKC):
            # V[tb] is consumed by window 0's deferred attn*V flush (due 0).
            fillers.push(0, lambda t=tb: proj_v(t))
        for jf in range(1, NCI):
            for blk in range(NQB):
                fillers.push(jf - 1, lambda j=jf, b=blk: proj_kq(KT[j], wk_sb, bk_sb, j, b, xk_blk[b]))
            fillers.push(jf - 1, lambda j=jf: proj_kq(QT[j], wq_sb, bq_sb, j, 0, xq_blk[0]))

        # Early windows fully defer attn*V (their V tiles are still being
        # projected) and drain fillers harder; steady state trails by 2.
        DEFER = {0: NG, 1: 8, 2: 5, 15: 1}
        POPS = {0: 3, 1: 2, 2: 2, 3: 2}

        # ---- main loop: n outer, j inner ----
        pending_finale = None
        for n in range(NQB):
            if n + 1 < NQB:
                nc.sync.dma_start(
                    out=xq_blk[n + 1], in_=xq_rr[:, :, QB * (n + 1) : QB * (n + 2)]
                )
            for j in range(NCI):
                w = 4 * n + j
                pf = attn_window(
                    n, j, fillers,
                    ov_defer=DEFER.get(w, OV_DEFER),
                    pops_per_slot=POPS.get(w, 1),
                    defer_finale=(w <= 14),
                    prev_finale=pending_finale,
                )
                pending_finale = pf
                if n + 1 < NQB:
                    fillers.push(
                        4 * (n + 1) + j - 1,
                        lambda jj=j, nn=n + 1: proj_kq(
                            QT[jj], wq_sb, bq_sb, jj, nn, xq_blk[nn]
                        ),
                    )
            for m in range(4 * n, 4 * n + 4):
                fillers.push(4 * (n + 1) + 2, lambda mm=m: out_proj_m(mm))
        fillers.drain_all()


def _emit_chain(tc, chain_in, chain_out):
    # Tiny passthrough used by the benchmark to serialize back-to-back NEFF
    # executions with a data dependency; ~2 DMAs, negligible cost.
    nc = tc.nc
    with tc.tile_pool(name="chp", bufs=1) as chp:
        ct = chp.tile([1, 1], F32, name="ct")
        nc.sync.dma_start(out=ct, in_=chain_in[:])
        nc.sync.dma_start(out=chain_out[:], in_=ct)


def build_nc():
    nc = bass.Bass(target_bir_lowering=False)
    xqT = nc.declare_dram_parameter("xqT", [D, S], F16, isOutput=False)
    xkT = nc.declare_dram_parameter("xkT", [D, S], F16, isOutput=False)
    xvT = nc.declare_dram_parameter("xvT", [D, S], F16, isOutput=False)
    wqT = nc.declare_dram_parameter("wqT", [D, CH], F16, isOutput=False)
    wkT = nc.declare_dram_parameter("wkT", [D, CH], F16, isOutput=False)
    wvT = nc.declare_dram_parameter("wvT", [D, CH], F16, isOutput=False)
    woT = nc.declare_dram_parameter("woT", [CH, D], F16, isOutput=False)
    bq = nc.declare_dram_parameter("bq", [CH], F32, isOutput=False)
    bk = nc.declare_dram_parameter("bk", [CH], F32, isOutput=False)
    bv = nc.declare_dram_parameter("bv", [CH], F32, isOutput=False)
    y = nc.declare_dram_parameter("y", [S, D], F16, isOutput=True)
    with tile.TileContext(nc) as tc:
        _emit(tc, xqT, xkT, xvT, wqT, wkT, wvT, woT, bq, bk, bv, y)
    nc.finalize()
    return nc


def make_in_maps(query, key, value, w_q, b_q, w_k, b_k, w_v, b_v, w_o, b_o):
    query = np.asarray(query, np.float32)
    key = np.asarray(key, np.float32)
    value = np.asarray(value, np.float32)
    w_q = np.asarray(w_q, np.float32)
    w_k = np.asarray(w_k, np.float32)
    w_v = np.asarray(w_v, np.float32)
    w_o = np.asarray(w_o, np.float32)
    in_maps = []
    for c in range(N_CORES):
        b, hh = divmod(c, 2)
        sl = slice(hh * CH, (hh + 1) * CH)
        in_maps.append(
            {
                "xqT": query[b].T.astype(np.float16),
                "xkT": key[b].T.astype(np.float16),
                "xvT": value[b].T.astype(np.float16),
                "wqT": w_q[sl].T.astype(np.float16),
                "wkT": w_k[sl].T.astype(np.float16),
                "wvT": w_v[sl].T.astype(np.float16),
                "woT": w_o[:, sl].T.astype(np.float16),
                "bq": np.ascontiguousarray(np.asarray(b_q, np.float32)[sl]),
                "bk": np.ascontiguousarray(np.asarray(b_k, np.float32)[sl]),
                "bv": np.ascontiguousarray(np.asarray(b_v, np.float32)[sl]),
            }
        )
    return in_maps


def run(trace=False, **inputs):
    global _NC_CACHE
    if _NC_CACHE is None:
        _NC_CACHE = build_nc()
    in_maps = make_in_maps(**inputs)
    res = run_bass_kernel_spmd(_NC_CACHE, in_maps, list(range(N_CORES)), trace=trace)
    b_o = np.asarray(inputs["b_o"], np.float32)
    y = np.empty((B, S, D), np.float32)
    for b in range(B):
        y[b] = (
            res.results[2 * b]["y"].astype(np.float32)
            + res.results[2 * b + 1]["y"].astype(np.float32)
            + b_o
        )
    return y, res


def kernel(**inputs):
    y, _ = run(trace=False, **inputs)
    return y



# revision 2
# speedup vs baseline: 1.0356x; 1.0356x over previous
"""Multi-head attention (B=4, S=2048, D=1024, H=16, Dk=64) on 8 trn2 NeuronCores.

Sharding: core c handles batch b = c//2 and head-half hh = c%2 (8 heads = 512
channels).  No collectives: each core produces a partial output projection
(sum over its 8 heads); the host adds the two partials per batch plus b_o.

Per-core kernel (fp16 operands, f32 PSUM accumulation), software-pipelined:
  - PSUM layout (8 banks): score ring 2 slots x [128, 2, 512] f32 (2 banks
    each; one t_k chunk x both heads per slot) + 2 banks of attn*V
    accumulators + 2 banks of projection accumulators (projp).  Projection /
    output-projection fillers use projp so they NEVER block the score ring -
    the exp stream on ScalarE is the near-critical resource (266us vs PE
    274us) and must not starve.
  - exp processes a whole ring slot (1024 wide) per instruction.
  - Window (n, j): 16 groups; group g = scores for t_k chunk g, both heads
    (2 matmuls, contract 64), one exp, and the attn*V for group g-OV_DEFER
    (8 matmuls of [128,65], accumulated into 2 packed PSUM banks with the
    fused ones-column rowsum).  Deadline-tagged fillers soak PE slack.
  - Window finale is split: PE flush of the last OV_DEFER groups runs at
    group 1 of the next window, the DVE normalize + XBAR transpose at group
    2, so PE/ACT never see a serial bubble at window boundaries.
  - DMA order puts the critical prefix (wk[j0], xk0, wq[j0], xq0) first so
    the exp stream starts at ~11us; window 0 projects KT[0] blocks 1-3
    just-in-time via pre-group hooks while xk1-3 stream in.
"""

import sys
from collections import deque

import numpy as np

if "/opt/trn_rl_repo" not in sys.path:
    sys.path.insert(0, "/opt/trn_rl_repo")

import concourse.bass as bass
import concourse.tile as tile
from concourse import mybir
from concourse.bass_utils import run_bass_kernel_spmd
import concourse.bass_utils as _bass_utils
import concourse.bass2jax as _bass2jax


def _legalize_bir_json(bir_json):
    """Walrus (this toolchain's codegen) encodes at most ONE attached sync
    wait per TPB instruction; Tile emits instructions with several.  Hoist
    all but the last wait of each non-DMA instruction into standalone
    EventSemaphore instructions right before it (same engine, so stream
    order preserves the wait semantics).  DMA waits ride in queue
    descriptors and are left untouched."""
    import json as _json

    d = _json.loads(bir_json)
    n = 0
    for fn in d.get("functions", []):
        for blk in fn.get("blocks", []):
            out = []
            for inst in blk.get("instructions", []):
                si = inst.get("sync_info")
                if si and si.get("on_wait") and len(si["on_wait"]) > 1:
                    waits = si["on_wait"]
                    for w in waits[:-1]:
                        n += 1
                        out.append(
                            {
                                "debug": inst.get("debug"),
                                "engine": inst["engine"],
                                "ins": [],
                                "outs": [],
                                "name": f"{inst['name']}-hw{n}",
                                "opcode": "EventSemaphore",
                                "sync_info": {"on_update": [], "on_wait": [w]},
                            }
                        )
                    si["on_wait"] = [waits[-1]]
                out.append(inst)
            blk["instructions"] = out
    return _json.dumps(d).encode()


_orig_compile_bir_kernel = _bass_utils.compile_bir_kernel


def _patched_compile_bir_kernel(bir_json, tmpdir, neff_name="file.neff"):
    return _orig_compile_bir_kernel(_legalize_bir_json(bir_json), tmpdir, neff_name)


_bass_utils.compile_bir_kernel = _patched_compile_bir_kernel
_bass2jax.compile_bir_kernel = _patched_compile_bir_kernel

F16 = mybir.dt.float16
F32 = mybir.dt.float32
EXP = mybir.ActivationFunctionType.Exp
PSUM = bass.MemorySpace.PSUM

B = 4          # batches (full problem)
S = 2048       # sequence length
D = 1024       # d_model
CH = 512       # channels per core (8 heads x 64)
NH = 8         # heads per core
DK = 64        # head dim
NDM = 8        # d_model chunks of 128
NCI = 4        # channel chunks of 128 (head pairs)
QB = 512       # t_q block width
NQB = S // QB  # 4
NKC = S // 128  # 16 t_k chunks
NG = NKC       # 16 groups per window (one t_k chunk, both heads)
OV_DEFER = 4   # attn*V trails exp by this many groups
N_CORES = 8

# ---- schedule knobs (tuned against TimelineSim) ----
# window 0 pre-group hooks: project KT[0] block b before this group
W0_KBLK_G = {3: 1, 7: 2, 11: 3}
# per-window attn*V defer / filler pops per slack slot
DEFER = {0: 6, 1: 5}
POPS = {0: 2, 1: 2, 2: 2, 3: 2}
# groups (of the next window) at which the previous window's finale parts run
FIN_G = (1, 2)

_NC_CACHE = None


class _Fillers:
    """Deadline-tagged PE work items woven into attention windows.

    `due` = index of the window by whose END the item must be emitted.
    Window w calls drain_due(w - 1) at its start and drain_due(w) before its
    finale; pop_one() opportunistically drains FIFO at slack slots."""

    def __init__(self):
        self.q = deque()  # (due_window, emit_fn)

    def push(self, due, fn):
        self.q.append((due, fn))

    def drain_due(self, w):
        while self.q and self.q[0][0] <= w:
            self.q.popleft()[1]()

    def pop_one(self):
        if self.q:
            self.q.popleft()[1]()

    def drain_all(self):
        while self.q:
            self.q.popleft()[1]()


def _emit(tc, xqT, xkT, xvT, wqT, wkT, wvT, woT, bq, bk, bv, y):
    nc = tc.nc

    with (
        tc.tile_pool(name="persist", bufs=1) as persist,
        tc.tile_pool(name="wp", bufs=1) as wp,
        tc.tile_pool(name="xkp", bufs=4) as xkp,
        tc.tile_pool(name="xqp", bufs=2) as xqp,
        tc.tile_pool(name="xvp", bufs=3) as xvp,
        tc.tile_pool(name="bigp", bufs=2, space=PSUM) as bigp,
        tc.tile_pool(name="ovp", bufs=2, space=PSUM) as ovp,
        tc.tile_pool(name="projp", bufs=2, space=PSUM) as projp,
        tc.tile_pool(name="ptp", bufs=OV_DEFER + 4) as ptp,
        tc.tile_pool(name="o2p", bufs=4) as o2p,
        tc.tile_pool(name="rrp", bufs=4) as rrp,
        tc.tile_pool(name="yp", bufs=2) as yp,
    ):
        QT = [persist.tile([128, S], F16, tag=f"qt{i}", name=f"qt{i}") for i in range(NCI)]
        KT = [persist.tile([128, S], F16, tag=f"kt{i}", name=f"kt{i}") for i in range(NCI)]
        V = [persist.tile([128, NH, DK + 1], F16, tag=f"v{t}", name=f"v{t}") for t in range(NKC)]
        OT = [persist.tile([128, NKC, 128], F16, tag=f"ot{i}", name=f"ot{i}") for i in range(NCI)]

        wo_sb = persist.tile([128, NCI, D], F16, tag="wo", name="wo")
        bq_sb = persist.tile([128, NCI], F32, tag="bq", name="bq")
        bk_sb = persist.tile([128, NCI], F32, tag="bk", name="bk")
        bv_bc = persist.tile([128, NH, DK + 1], F32, tag="bvb", name="bvb")

        wk_sb = wp.tile([128, NDM, CH], F16, tag="wk", name="wk")
        wq_sb = wp.tile([128, NDM, CH], F16, tag="wq", name="wq")
        wv_sb = wp.tile([128, NDM, CH], F16, tag="wv", name="wv")

        # ---- input loads.  SP queue order == issue order; the critical
        # prefix (wk[j0], xk0, wq[j0], xq0) comes first so window 0's exp
        # stream starts ~11us in, then xk/xv alternate with the leftover
        # weight columns so both the score stream and the V projections are
        # fed just-in-time. ----
        xk_rr = xkT[:].rearrange("(n p) s -> p n s", p=128)
        xq_rr = xqT[:].rearrange("(n p) s -> p n s", p=128)
        xv_rr = xvT[:].rearrange("(n p) s -> p n s", p=128)
        wk_rr = wkT[:].rearrange("(n p) c -> p n c", p=128)
        wq_rr = wqT[:].rearrange("(n p) c -> p n c", p=128)

        xk_blk = [xkp.tile([128, NDM, QB], F16, tag="xk", name=f"xk{b}") for b in range(NQB)]
        xq_blk = [xqp.tile([128, NDM, QB], F16, tag="xq", name=f"xq{b}") for b in range(NQB)]
        xv_blk = [xvp.tile([128, NDM, QB], F16, tag="xv", name=f"xv{b}") for b in range(NQB)]

        nc.sync.dma_start(out=wk_sb[:, :, 0:128], in_=wk_rr[:, :, 0:128])
        nc.sync.dma_start(out=bk_sb, in_=bk[:].rearrange("(n p) -> p n", p=128))
        nc.sync.dma_start(out=xk_blk[0], in_=xk_rr[:, :, 0:QB])
        nc.sync.dma_start(out=wq_sb[:, :, 0:128], in_=wq_rr[:, :, 0:128])
        nc.sync.dma_start(out=bq_sb, in_=bq[:].rearrange("(n p) -> p n", p=128))
        nc.sync.dma_start(out=xq_blk[0], in_=xq_rr[:, :, 0:QB])

        nc.sync.dma_start(out=wv_sb, in_=wvT[:].rearrange("(n p) c -> p n c", p=128))
        bv_r = bv[:].rearrange("(h d) -> h d", h=NH)
        nc.sync.dma_start(out=bv_bc[:, :, 0:DK], in_=bv_r.partition_broadcast(128))
        nc.vector.memset(bv_bc[:, :, DK : DK + 1], 0.0)

        nc.sync.dma_start(out=xv_blk[0], in_=xv_rr[:, :, 0:QB])
        nc.sync.dma_start(out=xk_blk[1], in_=xk_rr[:, :, QB : 2 * QB])
        nc.sync.dma_start(out=wk_sb[:, :, 128:CH], in_=wk_rr[:, :, 128:CH])
        nc.sync.dma_start(out=wq_sb[:, :, 128:CH], in_=wq_rr[:, :, 128:CH])
        nc.sync.dma_start(out=xv_blk[1], in_=xv_rr[:, :, QB : 2 * QB])
        nc.sync.dma_start(out=xk_blk[2], in_=xk_rr[:, :, 2 * QB : 3 * QB])
        nc.sync.dma_start(out=xv_blk[2], in_=xv_rr[:, :, 2 * QB : 3 * QB])
        nc.sync.dma_start(out=xk_blk[3], in_=xk_rr[:, :, 3 * QB : 4 * QB])
        nc.sync.dma_start(out=xv_blk[3], in_=xv_rr[:, :, 3 * QB : 4 * QB])
        nc.sync.dma_start(out=wo_sb, in_=woT[:].rearrange("(n p) d -> p n d", p=128))

        # ---- emission helpers ----
        def proj_ps():
            return projp.tile([128, QB], F32, tag="pj", name="pj")

        def proj_kq(dst, w_sb, bias_sb, j, blk, x_t):
            ps = proj_ps()
            for k in range(NDM):
                nc.tensor.matmul(
                    ps,
                    w_sb[:, k, 128 * j : 128 * (j + 1)],
                    x_t[:, k, :],
                    start=(k == 0),
                    stop=(k == NDM - 1),
                )
            nc.vector.tensor_scalar_add(
                out=dst[:, QB * blk : QB * (blk + 1)],
                in0=ps,
                scalar1=bias_sb[:, j : j + 1],
            )

        def proj_v(tb):
            x_t = xv_blk[tb // 4]
            c = tb % 4
            ps = proj_ps()
            for k in range(NDM):
                nc.tensor.matmul(
                    ps,
                    x_t[:, k, 128 * c : 128 * (c + 1)],
                    wv_sb[:, k, :],
                    start=(k == 0),
                    stop=(k == NDM - 1),
                )
            psv = ps.rearrange("p (h d) -> p h d", h=NH)
            nc.vector.tensor_add(out=V[tb][:, :, 0:DK], in0=psv, in1=bv_bc[:, :, 0:DK])
            nc.vector.memset(V[tb][:, :, DK : DK + 1], 1.0)

        y_r = y[:].rearrange("(a p) d -> a p d", p=128)

        def attn_window(n, j, fillers, ov_defer=OV_DEFER, pops_per_slot=1,
                        defer_finale=False, prev_finale=None, pre_ops=None):
            w = 4 * n + j
            fillers.drain_due(w - 1)
            ov = [
                ovp.tile([128, NQB, DK + 1], F32, tag="ov", name=f"ov{h}")
                for h in range(2)
            ]
            started = [False, False]
            pt_tiles = []

            def emit_ov(g):
                for h in range(2):
                    for c in range(NQB):
                        nc.tensor.matmul(
                            ov[h][:, c, :],
                            pt_tiles[g][:, h, 128 * c : 128 * (c + 1)],
                            V[g][:, 2 * j + h, :],
                            start=(not started[h]) and (c == 0),
                            stop=(g == NG - 1),
                            skip_group_check=True,
                        )
                    started[h] = True

            for g in range(NG):
                if pre_ops is not None and g in pre_ops:
                    pre_ops[g]()
                ps = bigp.tile([128, 2, QB], F32, tag="s", name="s")
                for h in range(2):
                    nc.tensor.matmul(
                        ps[:, h, :],
                        KT[j][64 * h : 64 * (h + 1), 128 * g : 128 * (g + 1)],
                        QT[j][64 * h : 64 * (h + 1), QB * n : QB * (n + 1)],
                        start=True,
                        stop=True,
                    )
                pt = ptp.tile([128, 2, QB], F16, tag="pt", name="pt")
                nc.scalar.activation(out=pt, in_=ps, func=EXP, scale=0.125)
                pt_tiles.append(pt)
                if prev_finale is not None:
                    for fi, fg in enumerate(FIN_G):
                        if g == fg and fi < len(prev_finale):
                            prev_finale[fi]()
                if g >= ov_defer:
                    emit_ov(g - ov_defer)
                if g >= 2:
                    for _ in range(pops_per_slot):
                        fillers.pop_one()

            def fin_flush():
                fillers.drain_due(w)
                for g in range(max(NG - ov_defer, 0), NG):
                    emit_ov(g)

            def fin_norm():
                r4 = []
                for h in range(2):
                    r = rrp.tile([128, NQB], F32, tag="r", name="r")
                    nc.vector.reciprocal(r, ov[h][:, :, DK])
                    r4.append(r)
                for c in range(NQB):
                    o2 = o2p.tile([128, 128], F16, tag="o2", name="o2")
                    for h in range(2):
                        nc.vector.tensor_scalar_mul(
                            out=o2[:, DK * h : DK * (h + 1)],
                            in0=ov[h][:, c, 0:DK],
                            scalar1=r4[h][:, c : c + 1],
                        )
                    nc.sync.dma_start_transpose(OT[j][:, NQB * n + c, :], o2)

            if defer_finale:
                return [fin_flush, fin_norm]
            fin_flush()
            fin_norm()
            return None

        def out_proj_m(m):
            y_sb = yp.tile([128, D], F16, tag="y", name="y")
            for half in range(2):
                ps = proj_ps()
                for jj in range(NCI):
                    nc.tensor.matmul(
                        ps,
                        OT[jj][:, m, :],
                        wo_sb[:, jj, QB * half : QB * (half + 1)],
                        start=(jj == 0),
                        stop=(jj == NCI - 1),
                    )
                nc.vector.tensor_copy(y_sb[:, QB * half : QB * (half + 1)], ps)
                # fire each half's store as soon as its copy lands, so the
                # final row's DMA doesn't wait for the whole tile
                nc.sync.dma_start(
                    out=y_r[m][:, QB * half : QB * (half + 1)],
                    in_=y_sb[:, QB * half : QB * (half + 1)],
                )

        # ---- prologue: KT[0] block 0 + QT[0] block 0 only; KT[0] blocks
        # 1-3 are projected just-in-time inside window 0 (pre-group hooks),
        # V and the other KT/QT projections are deadline-tagged fillers. ----
        proj_kq(KT[0], wk_sb, bk_sb, 0, 0, xk_blk[0])
        proj_kq(QT[0], wq_sb, bq_sb, 0, 0, xq_blk[0])

        w0_pre = {
            g: (lambda b=b: proj_kq(KT[0], wk_sb, bk_sb, 0, b, xk_blk[b]))
            for g, b in W0_KBLK_G.items()
        }

        fillers = _Fillers()
        for tb in range(NKC):
            fillers.push(0, lambda t=tb: proj_v(t))
        for jf in range(1, NCI):
            for blk in range(NQB):
                fillers.push(jf - 1, lambda j=jf, b=blk: proj_kq(KT[j], wk_sb, bk_sb, j, b, xk_blk[b]))
            fillers.push(jf - 1, lambda j=jf: proj_kq(QT[j], wq_sb, bq_sb, j, 0, xq_blk[0]))

        # ---- main loop: n outer, j inner ----
        pending_finale = None
        for n in range(NQB):
            if n + 1 < NQB:
                nc.sync.dma_start(
                    out=xq_blk[n + 1], in_=xq_rr[:, :, QB * (n + 1) : QB * (n + 2)]
                )
            for j in range(NCI):
                w = 4 * n + j
                pf = attn_window(
                    n, j, fillers,
                    ov_defer=DEFER.get(w, OV_DEFER),
                    pops_per_slot=POPS.get(w, 1),
                    defer_finale=(w <= 14),
                    prev_finale=pending_finale,
                    pre_ops=(w0_pre if w == 0 else None),
                )
                pending_finale = pf
                if n + 1 < NQB:
                    fillers.push(
                        4 * (n + 1) + j - 1,
                        lambda jj=j, nn=n + 1: proj_kq(
                            QT[jj], wq_sb, bq_sb, jj, nn, xq_blk[nn]
                        ),
                    )
            for m in range(4 * n, 4 * n + 4):
                fillers.push(4 * (n + 1) + 2, lambda mm=m: out_proj_m(mm))
        fillers.drain_all()


def _emit_chain(tc, chain_in, chain_out):
    # Tiny passthrough used by the benchmark to serialize back-to-back NEFF
    # executions with a data dependency; ~2 DMAs, negligible cost.
    nc = tc.nc
    with tc.tile_pool(name="chp", bufs=1) as chp:
        ct = chp.tile([1, 1], F32, name="ct")
        nc.sync.dma_start(out=ct, in_=chain_in[:])
        nc.sync.dma_start(out=chain_out[:], in_=ct)


def build_nc():
    nc = bass.Bass(target_bir_lowering=False)
    xqT = nc.declare_dram_parameter("xqT", [D, S], F16, isOutput=False)
    xkT = nc.declare_dram_parameter("xkT", [D, S], F16, isOutput=False)
    xvT = nc.declare_dram_parameter("xvT", [D, S], F16, isOutput=False)
    wqT = nc.declare_dram_parameter("wqT", [D, CH], F16, isOutput=False)
    wkT = nc.declare_dram_parameter("wkT", [D, CH], F16, isOutput=False)
    wvT = nc.declare_dram_parameter("wvT", [D, CH], F16, isOutput=False)
    woT = nc.declare_dram_parameter("woT", [CH, D], F16, isOutput=False)
    bq = nc.declare_dram_parameter("bq", [CH], F32, isOutput=False)
    bk = nc.declare_dram_parameter("bk", [CH], F32, isOutput=False)
    bv = nc.declare_dram_parameter("bv", [CH], F32, isOutput=False)
    y = nc.declare_dram_parameter("y", [S, D], F16, isOutput=True)
    with tile.TileContext(nc) as tc:
        _emit(tc, xqT, xkT, xvT, wqT, wkT, wvT, woT, bq, bk, bv, y)
    nc.finalize()
    return nc


def make_in_maps(query, key, value, w_q, b_q, w_k, b_k, w_v, b_v, w_o, b_o):
    query = np.asarray(query, np.float32)
    key = np.asarray(key, np.float32)
    value = np.asarray(value, np.float32)
    w_q = np.asarray(w_q, np.float32)
    w_k = np.asarray(w_k, np.float32)
    w_v = np.asarray(w_v, np.float32)
    w_o = np.asarray(w_o, np.float32)
    in_maps = []
    for c in range(N_CORES):
        b, hh = divmod(c, 2)
        sl = slice(hh * CH, (hh + 1) * CH)
        in_maps.append(
            {
                "xqT": query[b].T.astype(np.float16),
                "xkT": key[b].T.astype(np.float16),
                "xvT": value[b].T.astype(np.float16),
                "wqT": w_q[sl].T.astype(np.float16),
                "wkT": w_k[sl].T.astype(np.float16),
                "wvT": w_v[sl].T.astype(np.float16),
                "woT": w_o[:, sl].T.astype(np.float16),
                "bq": np.ascontiguousarray(np.asarray(b_q, np.float32)[sl]),
                "bk": np.ascontiguousarray(np.asarray(b_k, np.float32)[sl]),
                "bv": np.ascontiguousarray(np.asarray(b_v, np.float32)[sl]),
            }
        )
    return in_maps


def run(trace=False, **inputs):
    global _NC_CACHE
    if _NC_CACHE is None:
        _NC_CACHE = build_nc()
    in_maps = make_in_maps(**inputs)
    res = run_bass_kernel_spmd(_NC_CACHE, in_maps, list(range(N_CORES)), trace=trace)
    b_o = np.asarray(inputs["b_o"], np.float32)
    y = np.empty((B, S, D), np.float32)
    for b in range(B):
        y[b] = (
            res.results[2 * b]["y"].astype(np.float32)
            + res.results[2 * b + 1]["y"].astype(np.float32)
            + b_o
        )
    return y, res


def kernel(**inputs):
    y, _ = run(trace=False, **inputs)
    return y
